# revision 10
# baseline (speedup 1.0000x reference)
"""Trainium2 Bass kernel for 2D block-local multi-head attention.

Problem (hardcoded): x [1,128,48,64] -> 3x3 conv projections to q/k/v
(d_model=32, 8 heads, d_head=4), t2t local_attention_2d with
query_shape=(128,24), memory_flange=(8,8), combine heads, 3x3 output conv.

Structural facts exploited:
  * H=128, W=48, query blocks 128x24 -> exactly 2 blocks (nH=1, nW=2).
  * The memory flange is entirely zero padding, masked to -1e9 by the
    reference (exp -> exactly 0), so block b's effective key set is the
    static 128x32 strip of ORIGINAL pixels: cols [16b, 16b+32).
  * bf16 exp weights keep softmax exact enough.

Sharding: one head per NeuronCore, zero cross-core communication. Each
core computes q/k/v (full image), block-local attention, and a partial
output conv over its 4 head channels; host sums the 8 partial results.

Schedule (v2 - immediate AV):
  * Logits for tile (b,g,kt) -> two 512-col matmuls into a 7-unit PSUM
    ring (2KB banks).  exp reads the adjacent unit pair as one [128,1024]
    op; pair allocation rotates 2i mod 7 so a unit is reused 3.5 tiles
    later (> the exp+sem+matmul turnaround) -> no ring stalls.  The 1-in-7
    wraparound pair is split into two 512-col exp ops.
  * exp split ACT (table exp) / DVE (Schraudolph int16 trick) by a greedy
    load-balance over modeled per-op costs; drains/V'/norm/copies also
    greedy so both engines stay packed.
  * Immediate AV: right after exp(kt), 8 stationary-exp matmuls (one per
    128-query chunk) accumulate into a single shared PSUM bank (o5
    regions, granule-parity double buffered; 8 interleaved per-region
    accumulation chains, skip_group_check).  ex is a small 6-slot ring;
    the old granule-sized exp buffers are gone.  AV lags exp by one tile
    so PE's in-order queue never blocks the next logits.
  * Conv phase: 5-tap packed conv matmuls; PSUM drains (bias add) on
    ACT/DVE greedy; k/v block strips extracted with one merged DMA per
    (block, 4-chunk group); V' transposes+copies interleaved; early exp
    of both blocks' granule 0 starts at chunk 5.
  * Per-granule softmax: one reciprocal over the 8 denominators, 8 small
    scaled copies into o4nb, then a per-granule flush: full-o4nb XBAR
    transpose (only fresh rows used), partition-collapse shift DMAs into
    obT, and 9 tap DMAs (on gpsimd SWDGE to bypass HWDGE) building the
    36-partition stacked image for the output conv.
  * Tail: the last granule's flush uses a PE transpose (id128) + engine
    copy instead of the XBAR DMA, shifts/taps spread over 4 DMA queues;
    output conv = 16 single-matmul chunks pipelined over the free ring
    units with drains split ACT/DVE.
"""

import numpy as np

H, W, CIN, DM, NH, DH = 128, 48, 64, 32, 8, 4
HP, WP = H + 2, W + 2          # padded spatial dims for 3x3 SAME conv
PADN = HP * WP + 4             # padded flat buffer size (+4 tail slack)
NPIX = H * W                   # 6144
QW, KW = 24, 32                # per-block query/key column widths
NQ = H * QW                    # 3072 queries per block
NK = H * KW                    # 4096 keys per block
NKT = 32                       # key tiles (128 keys each) per block
G = 1024                       # query granule (exp tile width)
NG = NQ // G                   # 3
QC = 128                       # AV query chunk (psum partitions)
NQC = G // QC                  # 8 chunks per granule
CHUNK_ROWS = 8                 # conv output rows per matmul chunk
NCHUNK = H // CHUNK_ROWS       # 16
CN = CHUNK_ROWS * WP           # conv matmul free size, 400
OOF = 4                        # oo36 head slack (tap shifts can hit -1)
NU = 512                       # psum ring unit = one 2KB bank of f32
EXS = 6                        # ex ring slots

SCH_A = 128.0 / float(np.log(2.0))   # Schraudolph scale (bf16 pattern)
SCH_B = 127.0 * 128.0                # Schraudolph exponent-bias offset

# modeled per-op engine costs (ns) for the greedy ACT/DVE balance
C_EXP_A, C_EXP_D = 1038.0, 1192.0
C_EXPH_A, C_EXPH_D = 611.0, 658.0    # 512-col half (wrap pair)
C_DRAIN_A, C_DRAIN_D = 505.0, 525.0
C_VP_A, C_VP_D = 212.0, 258.0
C_NORM_A, C_NORM_D = 190.0, 130.0
C_OC_A, C_OC_D = 505.0, 525.0

_cached = {}


def _build_nc():
    import concourse.bacc as bacc
    import concourse.tile as tile
    import concourse.mybir as mybir

    f32 = mybir.dt.float32
    bf16 = mybir.dt.bfloat16
    i16 = mybir.dt.int16
    AF = mybir.ActivationFunctionType
    ALU = mybir.AluOpType

    nc = bacc.Bacc("TRN2", target_bir_lowering=False)

    xx_d = nc.dram_tensor("xx", [128, PADN], bf16, kind="ExternalInput")
    xx2_d = nc.dram_tensor("xx2", [128, PADN], bf16, kind="ExternalInput")
    wqkv_d = nc.dram_tensor("wqkv", [128, 5 * 12], bf16, kind="ExternalInput")
    bias_d = nc.dram_tensor("bias12", [12, 1], f32, kind="ExternalInput")
    wo36_d = nc.dram_tensor("wo36", [36, 64], bf16, kind="ExternalInput")
    id8_d = nc.dram_tensor("id8", [8, 8], bf16, kind="ExternalInput")
    id128_d = nc.dram_tensor("id128", [128, 128], bf16, kind="ExternalInput")
    outp_d = nc.dram_tensor("outp", [CIN, NPIX], f32, kind="ExternalOutput")

    # greedy engine-load balance (build-time static schedule)
    load = {"A": 0.0, "D": 0.0}

    def pick(cA, cD):
        if load["A"] + cA <= load["D"] + cD:
            load["A"] += cA
            return "A"
        load["D"] += cD
        return "D"

    with tile.TileContext(nc) as tc:
        with tc.tile_pool(name="main", bufs=1) as mp:
            wqkv = mp.tile([128, 5 * 12], bf16)
            bias12 = mp.tile([12, 1], f32)
            wo36 = mp.tile([36, 64], bf16)
            id8 = mp.tile([8, 8], bf16)
            id128 = mp.tile([128, 128], bf16)
            kvTb = mp.tile([8, 2 * NK], bf16)    # k rows 0:4, v rows 4:8
            qb = mp.tile([DH, 2 * NQ], bf16)     # block-contiguous queries
            vp = mp.tile([128, 2 * NKT * 5], bf16)  # V': 4 v cols + 1.0
            obT = mp.tile([DH, 2 * NQ], bf16)    # normalized o^T, block-major
            oo36 = mp.tile([36, OOF + PADN], bf16)  # 9-tap stacked o^T
            ex = mp.tile([128, EXS * G], bf16)   # exp ring
            o4nb = [mp.tile([128, 128], bf16, name=f"o4nb{b}") for b in range(2)]
            dn = mp.tile([128, 16], f32)         # reciprocal denominators
            actwarm = mp.tile([128, 1], f32)
            pewarm = mp.tile([DH, 512], bf16)
            warmsrc = mp.tile([128, 1], f32)

            nc.vector.memset(warmsrc[:], -30.0)
            # dummy exp at t~0 pulls the ACT exp-table load off the
            # critical path
            nc.scalar.activation(actwarm[:], warmsrc[:], AF.Exp)
            nc.vector.memset(vp[:], 1.0)
            nc.gpsimd.memset(oo36[:], 0.0)

            nc.sync.dma_start(wqkv[:], wqkv_d.ap())
            nc.sync.dma_start(bias12[:], bias_d.ap())
            nc.scalar.dma_start(id8[:], id8_d.ap())
            nc.scalar.dma_start(id128[:], id128_d.ap())
            nc.scalar.dma_start(wo36[:], wo36_d.ap())

            qb_v = qb[:].rearrange("p (b h w) -> p b h w", b=2, w=QW)
            obv = obT[:].rearrange("p (b r w) -> p b r w", b=2, w=QW)

            with tc.tile_pool(name="psA", bufs=1, space="PSUM") as psA:
                big = psA.tile([128, 7 * NU], f32)   # 7 ring units
                o5b = psA.tile([128, NU], f32)       # o5 regions + scratch

                ring = {"i": 0, "units": [0, 1, 2, 3, 4]}
                exs = {"i": 0}
                pend_av = []   # (b, g, kt, slot, gidx) AV lag-1 queue

                def alloc_pair():
                    n = len(ring["units"])
                    u0 = ring["units"][ring["i"] % n]
                    u1 = ring["units"][(ring["i"] + 1) % n]
                    ring["i"] += 2
                    return u0, u1

                def emit_exp(b, g, kt, gidx):
                    u0, u1 = alloc_pair()
                    q0 = b * NQ + g * G
                    for j, u in ((0, u0), (1, u1)):
                        nc.tensor.matmul(
                            big[:, u * NU:(u + 1) * NU],
                            kvTb[0:4, b * NK + 128 * kt:b * NK + 128 * (kt + 1)],
                            qb[:, q0 + 512 * j:q0 + 512 * (j + 1)],
                            start=True, stop=True,
                        )
                    s = exs["i"] % EXS
                    exs["i"] += 1

                    def one_exp(src, dst, cA, cD):
                        if pick(cA, cD) == "A":
                            nc.scalar.activation(dst, src, AF.Exp)
                        else:
                            nc.vector.tensor_scalar(
                                out=dst.bitcast(i16), in0=src,
                                scalar1=SCH_A, scalar2=SCH_B,
                                op0=ALU.mult, op1=ALU.add,
                            )

                    if u1 == u0 + 1:
                        one_exp(big[:, u0 * NU:u0 * NU + 2 * NU],
                                ex[:, s * G:(s + 1) * G], C_EXP_A, C_EXP_D)
                    else:  # wraparound pair: two 512-col ops
                        for j, u in ((0, u0), (1, u1)):
                            one_exp(big[:, u * NU:(u + 1) * NU],
                                    ex[:, s * G + 512 * j:s * G + 512 * (j + 1)],
                                    C_EXPH_A, C_EXPH_D)
                    pend_av.append((b, g, kt, s, gidx))
                    if len(pend_av) > 1:
                        emit_av(*pend_av.pop(0))

                def emit_av(b, g, kt, s, gidx):
                    reg = 40 * (gidx % 2)
                    for qc in range(NQC):
                        nc.tensor.matmul(
                            o5b[:, reg + 5 * qc:reg + 5 * qc + 5],
                            ex[:, s * G + qc * QC:s * G + (qc + 1) * QC],
                            vp[:, (b * NKT + kt) * 5:(b * NKT + kt + 1) * 5],
                            start=(kt == 0), stop=(kt == NKT - 1),
                            skip_group_check=True,
                        )

                def drain_avs():
                    while pend_av:
                        emit_av(*pend_av.pop(0))

                def emit_norm(b, g, gidx):
                    reg = 40 * (gidx % 2)
                    o5v = o5b[:, reg:reg + 40].rearrange(
                        "p (q c) -> p q c", c=5)
                    dreg = 8 * (gidx % 2)
                    dnv = dn[:, dreg:dreg + 8].rearrange(
                        "p (q c) -> p q c", c=1)
                    nc.vector.reciprocal(dnv, o5v[:, :, 4:5])
                    for qc in range(NQC):
                        dst = o4nb[b][:, 32 * g + 4 * qc:32 * g + 4 * qc + 4]
                        src = o5b[:, reg + 5 * qc:reg + 5 * qc + 4]
                        dcol = dn[:, dreg + qc:dreg + qc + 1]
                        if pick(C_NORM_A, C_NORM_D) == "A":
                            nc.scalar.mul(dst, src, dcol)
                        else:
                            nc.vector.tensor_scalar(
                                out=dst, in0=src, scalar1=dcol,
                                scalar2=None, op0=ALU.mult)

                def emit_taps(b, g, r_lo, r_hi, queues):
                    qi = 0
                    for dh in range(3):
                        for dw in range(3):
                            t = 3 * dh + dw
                            rr0 = max(r_lo, 1) if dh == 2 else r_lo
                            nr = r_hi - rr0
                            if nr <= 0:
                                continue
                            off = OOF + (1 + rr0 - dh) * WP + QW * b + 1 - dw
                            dst = oo36[4 * t:4 * t + 4, off:off + nr * WP]
                            dst = dst.rearrange(
                                "p (r w) -> p r w", w=WP)[:, :, 0:QW]
                            queues[qi % len(queues)].dma_start(
                                dst, obv[:, b, rr0:rr0 + nr])
                            qi += 1

                ROW_END = [(g + 1) * G // QW for g in range(NG)]  # 42, 85, 128

                def emit_flush(b, g, st):
                    # full-o4nb XBAR transpose; only fresh rows consumed
                    nc.sync.dma_start_transpose(st[:], o4nb[b][:])
                    dst = obT[0:4, b * NQ + g * G:b * NQ + (g + 1) * G]
                    dstv = dst.rearrange("p (c q) -> p c q", q=QC)
                    for d in range(4):
                        nc.sync.dma_start(
                            dstv[d:d + 1], st[32 * g + d:32 * g + 32:4, 0:QC])
                    r_lo = ROW_END[g - 1] if g > 0 else 0
                    emit_taps(b, g, r_lo, ROW_END[g], [nc.gpsimd])

                # ---- conv + early attention ----
                with tc.tile_pool(name="cvp", bufs=1) as cvp:
                    xx = cvp.tile([128, PADN], bf16)
                    xx2 = cvp.tile([128, PADN], bf16)
                    qkvT = cvp.tile([12, NPIX], bf16)
                    xx_ap = xx_d.ap()
                    xx2_ap = xx2_d.ap()
                    for q4 in range(4):
                        s4 = (PADN // 4) * q4
                        e4 = PADN if q4 == 3 else (PADN // 4) * (q4 + 1)
                        nc.sync.dma_start(xx[:, s4:e4], xx_ap[:, s4:e4])
                        nc.scalar.dma_start(xx2[:, s4:e4], xx2_ap[:, s4:e4])

                    # dummy matmuls burn the cost model's 3us p-state ramp
                    # during the DMA-in window (result unused)
                    nc.vector.memset(pewarm[:], 1.0)
                    for _ in range(5):
                        nc.tensor.matmul(big[0:DH, 0:512], pewarm[:, 0:DH],
                                         pewarm[:], start=True, stop=True)

                    qk_v = qkvT[:].rearrange("p (h w) -> p h w", w=W)
                    qT_v = qkvT[0:4, :].rearrange("p (h w) -> p h w", w=W)

                    # early-exp schedule: group gi (kt 8gi..8gi+8, both
                    # blocks) available from chunk 4gi+5; queries from ci>=5
                    early = {ci: [] for ci in range(NCHUNK)}
                    pool_tiles = []
                    issued = {(0, 0): set(), (1, 0): set()}
                    for kt in range(NKT):
                        for b in range(2):
                            pool_tiles.append((b, kt))
                    pti = 0
                    for ci in range(5, NCHUNK):
                        cap = 2 if ci < 12 else 4
                        while cap > 0 and pti < len(pool_tiles):
                            b, kt = pool_tiles[pti]
                            if ci < 4 * (kt // 8) + 5:
                                break
                            early[ci].append((b, kt))
                            issued[(b, 0)].add(kt)
                            pti += 1
                            cap -= 1

                    for ci in range(NCHUNK):
                        u = 5 + (ci % 2)
                        ps = big[0:12, u * NU:u * NU + CN]
                        f0 = ci * CHUNK_ROWS * WP
                        for dh in range(3):
                            nc.tensor.matmul(
                                ps, wqkv[:, 12 * dh:12 * (dh + 1)],
                                xx[:, f0 + dh * WP:f0 + dh * WP + CN],
                                start=(dh == 0), stop=False,
                                skip_group_check=True,
                            )
                        nc.tensor.matmul(
                            ps, wqkv[:, 36:48], xx2[:, f0:f0 + CN],
                            start=False, stop=False, skip_group_check=True,
                        )
                        nc.tensor.matmul(
                            ps, wqkv[0:CIN, 48:60],
                            xx[0:CIN, f0 + 2 * WP + 2:f0 + 2 * WP + 2 + CN],
                            start=False, stop=True, skip_group_check=True,
                        )
                        # bias add + junk-column drop (cast to bf16)
                        psv = ps.rearrange("p (r c) -> p r c", c=WP)
                        dst = qkvT[:, ci * CHUNK_ROWS * W:
                                   (ci + 1) * CHUNK_ROWS * W]
                        if pick(C_DRAIN_A, C_DRAIN_D) == "A":
                            nc.scalar.add(dst, psv[:, :, 0:W], bias12[:])
                        else:
                            nc.vector.tensor_scalar_add(
                                dst, psv[:, :, 0:W], bias12[:])
                        r0 = ci * CHUNK_ROWS
                        rr = slice(r0, r0 + CHUNK_ROWS)
                        for b in range(2):
                            nc.gpsimd.tensor_copy(
                                qb_v[:, b, rr],
                                qT_v[:, rr, QW * b:QW * b + QW])
                        if ci % 4 == 3:
                            gi = ci // 4
                            rsl = slice(32 * gi, 32 * gi + 32)
                            for b in range(2):
                                dsl = slice(b * NK + 1024 * gi,
                                            b * NK + 1024 * (gi + 1))
                                nc.sync.dma_start(
                                    kvTb[:, dsl],
                                    qk_v[4:12, rsl, 16 * b:16 * b + KW])
                            # V' build for this group's 8 key tiles x 2
                            # blocks: PE transposes into the spare tail of
                            # the cps unit, engine copy to vp
                            tgu = 5 + (ci % 2)
                            tg = big[:, tgu * NU + 416:tgu * NU + 448]
                            tg = tg.bitcast(bf16)   # [128, 64]
                            for b in range(2):
                                for j in range(8):
                                    kt = 8 * gi + j
                                    nc.tensor.matmul(
                                        tg[:, 8 * j:8 * j + 8],
                                        kvTb[0:8, b * NK + 128 * kt:
                                             b * NK + 128 * (kt + 1)],
                                        id8[:], is_transpose=True,
                                        skip_group_check=True,
                                    )
                                dstv = vp[:].rearrange(
                                    "p (t c) -> p t c", c=5)
                                srcv = tg.rearrange("p (t c) -> p t c", c=8)
                                d0 = b * NKT + 8 * gi
                                if pick(C_VP_A, C_VP_D) == "A":
                                    nc.scalar.copy(
                                        dstv[:, d0:d0 + 8, 0:4],
                                        srcv[:, :, 4:8])
                                else:
                                    nc.vector.tensor_copy(
                                        dstv[:, d0:d0 + 8, 0:4],
                                        srcv[:, :, 4:8])
                        for b, kt in early[ci]:
                            emit_exp(b, 0, kt, b)

                # ---- steady attention ----
                ring["units"] = [0, 1, 2, 3, 4, 5, 6]
                ring["i"] = 0
                order = [(0, 0), (1, 0), (0, 1), (1, 1), (0, 2), (1, 2)]
                with tc.tile_pool(name="stp", bufs=2) as stp:
                    for gidx, (b, g) in enumerate(order):
                        done = issued.get((b, g), set())
                        for kt in range(NKT):
                            if kt in done:
                                continue
                            emit_exp(b, g, kt, gidx)
                        drain_avs()
                        emit_norm(b, g, gidx)
                        if gidx < 5:
                            st = stp.tile([128, 128], bf16, tag="st")
                            emit_flush(b, g, st)

                    # tail flush for (1,2): PE transpose + engine copy
                    # (lower latency than the XBAR DMA), shifts/taps spread
                    # over all four DMA queues
                    stt = stp.tile([128, 128], bf16, tag="st")
                    tg2 = o5b[0:32, 128:192].bitcast(bf16)   # [32, 128]
                    nc.tensor.matmul(tg2, o4nb[1][:, 64:96], id128[:],
                                     is_transpose=True, skip_group_check=True)
                    if pick(300.0, 260.0) == "A":
                        nc.scalar.copy(stt[0:32, :], tg2)
                    else:
                        nc.vector.tensor_copy(stt[0:32, :], tg2)
                    dst = obT[0:4, NQ + 2 * G:NQ + 3 * G]
                    dstv = dst.rearrange("p (c q) -> p c q", q=QC)
                    tailq = [nc.sync, nc.scalar, nc.gpsimd]
                    for d in range(4):
                        tailq[d % len(tailq)].dma_start(
                            dstv[d:d + 1], stt[d:32:4, 0:QC])
                    emit_taps(1, 2, ROW_END[1], ROW_END[2], tailq)

                # ---- output conv ----
                outp_ap = outp_d.ap()
                with tc.tile_pool(name="ost", bufs=3) as ost:
                    for c2 in range(NCHUNK // 2):
                        stage = ost.tile([CIN, 2 * CHUNK_ROWS * W], f32,
                                         tag="ost")
                        for half in range(2):
                            ci = 2 * c2 + half
                            u = ci % 7
                            ps = big[0:CIN, u * NU:u * NU + CN]
                            f0 = OOF + ci * CHUNK_ROWS * WP
                            nc.tensor.matmul(
                                ps, wo36[:], oo36[:, f0:f0 + CN],
                                start=True, stop=True, skip_group_check=True,
                            )
                            psv = ps.rearrange("p (r c) -> p r c", c=WP)
                            dst = stage[:, half * CHUNK_ROWS * W:
                                        (half + 1) * CHUNK_ROWS * W]
                            if pick(C_OC_A, C_OC_D) == "A":
                                nc.scalar.copy(dst, psv[:, :, 0:W])
                            else:
                                nc.vector.tensor_copy(dst, psv[:, :, 0:W])
                        nc.sync.dma_start(
                            outp_ap[:, 2 * c2 * CHUNK_ROWS * W:
                                    (2 * c2 + 2) * CHUNK_ROWS * W],
                            stage[:])

    nc.compile()
    return nc


def _prep_inputs(x, wq, bq, wk, bk, wv, bv, wo):
    f32 = np.float32
    x = np.ascontiguousarray(np.asarray(x, f32))
    scale = f32(DH) ** -0.5

    bf = ml_bf16()
    xx = np.zeros((128, PADN), np.float32)
    xv = xx[:CIN, :HP * WP].reshape(CIN, HP, WP)
    xv[:, 1:1 + H, 1:1 + W] = x[0].transpose(2, 0, 1)
    xx[CIN:, :PADN - 1] = xx[:CIN, 1:]
    xx2 = np.zeros((128, PADN), np.float32)
    xx2[:CIN, :PADN - 2] = xx[:CIN, 2:]
    xx2[CIN:, :PADN - (WP + 2)] = xx[:CIN, WP + 2:]
    xx2 = xx2.astype(bf)
    xx = xx.astype(bf)

    wq = np.asarray(wq, f32) * scale
    bq = np.asarray(bq, f32) * scale
    wk = np.asarray(wk, f32)
    bk = np.asarray(bk, f32)
    wv = np.asarray(wv, f32)
    bv = np.asarray(bv, f32)
    wo = np.asarray(wo, f32)

    id8 = np.eye(8, dtype=bf)
    id128 = np.eye(128, dtype=bf)
    in_maps = []
    for h in range(NH):
        sl = slice(4 * h, 4 * h + 4)
        wqkv = np.zeros((128, 5, 12), f32)
        for dh in range(3):
            for p, dw in ((0, 0), (1, 1)):   # pair slots on partition halves
                wqkv[64 * p:64 * p + CIN, dh, 0:4] = wq[dh, dw, :, sl]
                wqkv[64 * p:64 * p + CIN, dh, 4:8] = wk[dh, dw, :, sl]
                wqkv[64 * p:64 * p + CIN, dh, 8:12] = wv[dh, dw, :, sl]
        for p, dh in ((0, 0), (1, 1)):       # (0,2)+(1,2) pair on xx2 halves
            wqkv[64 * p:64 * p + CIN, 3, 0:4] = wq[dh, 2, :, sl]
            wqkv[64 * p:64 * p + CIN, 3, 4:8] = wk[dh, 2, :, sl]
            wqkv[64 * p:64 * p + CIN, 3, 8:12] = wv[dh, 2, :, sl]
        wqkv[:CIN, 4, 0:4] = wq[2, 2, :, sl]
        wqkv[:CIN, 4, 4:8] = wk[2, 2, :, sl]
        wqkv[:CIN, 4, 8:12] = wv[2, 2, :, sl]
        bias12 = np.concatenate([bq[sl], bk[sl], bv[sl]]).reshape(12, 1)
        wo36 = np.zeros((36, 64), f32)
        for dh in range(3):
            for dw in range(3):
                t = 3 * dh + dw
                wo36[4 * t:4 * t + 4, :] = wo[dh, dw, sl, :]
        in_maps.append({
            "xx": xx,
            "bias12": np.ascontiguousarray(bias12.astype(f32)),
            "wqkv": np.ascontiguousarray(wqkv.reshape(128, 5 * 12).astype(bf)),
            "xx2": xx2,
            "wo36": np.ascontiguousarray(wo36.astype(bf)),
            "id8": id8,
            "id128": id128,
        })
    return in_maps


def ml_bf16():
    import ml_dtypes
    return ml_dtypes.bfloat16


def _run(in_maps, trace=False, trace_cores=None):
    from concourse.bass_utils import run_bass_kernel_spmd

    if "nc" not in _cached:
        _cached["nc"] = _build_nc()
    return run_bass_kernel_spmd(
        _cached["nc"], in_maps, core_ids=list(range(NH)),
        trace=trace, trace_cores=trace_cores,
    )


def kernel(x, wq, bq, wk, bk, wv, bv, wo):
    in_maps = _prep_inputs(x, wq, bq, wk, bk, wv, bv, wo)
    res = _run(in_maps)
    acc = np.zeros((CIN, NPIX), np.float64)
    for r in res.results:
        acc += r["outp"].astype(np.float64)
    out = acc.astype(np.float32).reshape(CIN, H, W).transpose(1, 2, 0)
    return out[None]


# revision 11
# speedup vs baseline: 2.5157x; 2.5157x over previous
"""Trainium2 Bass kernel for 2D block-local multi-head attention.

Problem (hardcoded): x [1,128,48,64] -> 3x3 conv projections to q/k/v
(d_model=32, 8 heads, d_head=4), t2t local_attention_2d with
query_shape=(128,24), memory_flange=(8,8), combine heads, 3x3 output conv.

Structural facts exploited:
  * H=128, W=48, query blocks 128x24 -> exactly 2 blocks (nH=1, nW=2).
  * The memory flange is entirely zero padding, masked to -1e9 by the
    reference (exp -> exactly 0), so block b's effective key set is the
    static 128x32 strip of ORIGINAL pixels: cols [16b, 16b+32).
  * bf16 exp weights keep softmax exact enough.

Sharding: one head per NeuronCore, zero cross-core communication. Each
core computes q/k/v (full image), block-local attention, and a partial
output conv over its 4 head channels; host sums the 8 partial results.

Schedule (v2 - immediate AV):
  * Logits for tile (b,g,kt) -> two 512-col matmuls into a 7-unit PSUM
    ring (2KB banks).  exp reads the adjacent unit pair as one [128,1024]
    op; pair allocation rotates 2i mod 7 so a unit is reused 3.5 tiles
    later (> the exp+sem+matmul turnaround) -> no ring stalls.  The 1-in-7
    wraparound pair is split into two 512-col exp ops.
  * exp split ACT (table exp) / DVE (Schraudolph int16 trick) by a greedy
    load-balance over modeled per-op costs; drains/V'/norm/copies also
    greedy so both engines stay packed.
  * Immediate AV: right after exp(kt), 8 stationary-exp matmuls (one per
    128-query chunk) accumulate into a single shared PSUM bank (o5
    regions, granule-parity double buffered; 8 interleaved per-region
    accumulation chains, skip_group_check).  ex is a small 6-slot ring;
    the old granule-sized exp buffers are gone.  AV lags exp by one tile
    so PE's in-order queue never blocks the next logits.
  * Conv phase: 5-tap packed conv matmuls; PSUM drains (bias add) on
    ACT/DVE greedy; k/v block strips extracted with one merged DMA per
    (block, 4-chunk group); V' transposes+copies interleaved; early exp
    of both blocks' granule 0 starts at chunk 5.
  * Per-granule softmax: one reciprocal over the 8 denominators, 8 small
    scaled copies into o4nb, then a per-granule flush: full-o4nb XBAR
    transpose (only fresh rows used), partition-collapse shift DMAs into
    obT, and 9 tap DMAs (on gpsimd SWDGE to bypass HWDGE) building the
    36-partition stacked image for the output conv.
  * Tail: the last granule's flush uses a PE transpose (id128) + engine
    copy instead of the XBAR DMA, shifts/taps spread over 4 DMA queues;
    output conv = 16 single-matmul chunks pipelined over the free ring
    units with drains split ACT/DVE.
"""

import numpy as np

H, W, CIN, DM, NH, DH = 128, 48, 64, 32, 8, 4
HP, WP = H + 2, W + 2          # padded spatial dims for 3x3 SAME conv
PADN = HP * WP + 4             # padded flat buffer size (+4 tail slack)
NPIX = H * W                   # 6144
QW, KW = 24, 32                # per-block query/key column widths
NQ = H * QW                    # 3072 queries per block
NK = H * KW                    # 4096 keys per block
NKT = 32                       # key tiles (128 keys each) per block
G = 1024                       # query granule (exp tile width)
NG = NQ // G                   # 3
QC = 128                       # AV query chunk (psum partitions)
NQC = G // QC                  # 8 chunks per granule
CHUNK_ROWS = 8                 # conv output rows per matmul chunk
NCHUNK = H // CHUNK_ROWS       # 16
CN = CHUNK_ROWS * WP           # conv matmul free size, 400
OOF = 4                        # oo36 head slack (tap shifts can hit -1)
NU = 512                       # psum ring unit = one 2KB bank of f32
EXS = 6                        # ex ring slots

SCH_A = 128.0 / float(np.log(2.0))   # Schraudolph scale (bf16 pattern)
SCH_B = 127.0 * 128.0                # Schraudolph exponent-bias offset

# modeled per-op engine costs (ns) for the greedy ACT/DVE balance
C_EXP_A, C_EXP_D = 1038.0, 1192.0
C_EXPH_A, C_EXPH_D = 611.0, 658.0    # 512-col half (wrap pair)
C_DRAIN_A, C_DRAIN_D = 505.0, 525.0
C_VP_A, C_VP_D = 212.0, 258.0
C_NORM_A, C_NORM_D = 190.0, 130.0
C_OC_A, C_OC_D = 505.0, 525.0

_cached = {}


def _build_nc():
    import concourse.bacc as bacc
    import concourse.tile as tile
    import concourse.mybir as mybir

    f32 = mybir.dt.float32
    bf16 = mybir.dt.bfloat16
    i16 = mybir.dt.int16
    AF = mybir.ActivationFunctionType
    ALU = mybir.AluOpType

    nc = bacc.Bacc("TRN2", target_bir_lowering=False)

    xx_d = nc.dram_tensor("xx", [128, PADN], bf16, kind="ExternalInput")
    xx2_d = nc.dram_tensor("xx2", [128, PADN], bf16, kind="ExternalInput")
    wqkv_d = nc.dram_tensor("wqkv", [128, 5 * 12], bf16, kind="ExternalInput")
    bias_d = nc.dram_tensor("bias12", [12, 1], f32, kind="ExternalInput")
    wo36_d = nc.dram_tensor("wo36", [36, 64], bf16, kind="ExternalInput")
    id8_d = nc.dram_tensor("id8", [8, 8], bf16, kind="ExternalInput")
    id128_d = nc.dram_tensor("id128", [128, 128], bf16, kind="ExternalInput")
    outp_d = nc.dram_tensor("outp", [CIN, NPIX], f32, kind="ExternalOutput")

    # greedy engine-load balance (build-time static schedule)
    load = {"A": 0.0, "D": 0.0}

    def pick(cA, cD):
        if load["A"] + cA <= load["D"] + cD:
            load["A"] += cA
            return "A"
        load["D"] += cD
        return "D"

    with tile.TileContext(nc) as tc:
        with tc.tile_pool(name="main", bufs=1) as mp:
            wqkv = mp.tile([128, 5 * 12], bf16)
            bias12 = mp.tile([12, 1], f32)
            wo36 = mp.tile([36, 64], bf16)
            id8 = mp.tile([8, 8], bf16)
            id128 = mp.tile([128, 128], bf16)
            kvTb = mp.tile([8, 2 * NK], bf16)    # k rows 0:4, v rows 4:8
            qb = mp.tile([DH, 2 * NQ], bf16)     # block-contiguous queries
            vp = mp.tile([128, 2 * NKT * 5], bf16)  # V': 4 v cols + 1.0
            obT = mp.tile([DH, 2 * NQ], bf16)    # normalized o^T, block-major
            oo36 = mp.tile([36, OOF + PADN], bf16)  # 9-tap stacked o^T
            o4nb = [mp.tile([128, 128], bf16, name=f"o4nb{b}") for b in range(2)]
            dn = mp.tile([128, 16], f32)         # reciprocal denominators
            actwarm = mp.tile([128, 1], f32)
            pewarm = mp.tile([DH, 512], bf16)
            warmsrc = mp.tile([128, 1], f32)

            nc.vector.memset(warmsrc[:], -30.0)
            # dummy exp at t~0 pulls the ACT exp-table load off the
            # critical path
            nc.scalar.activation(actwarm[:], warmsrc[:], AF.Exp)
            nc.vector.memset(vp[:], 1.0)
            nc.gpsimd.memset(oo36[:], 0.0)

            nc.sync.dma_start(wqkv[:], wqkv_d.ap())
            nc.sync.dma_start(bias12[:], bias_d.ap())
            nc.scalar.dma_start(id8[:], id8_d.ap())
            nc.scalar.dma_start(id128[:], id128_d.ap())
            nc.scalar.dma_start(wo36[:], wo36_d.ap())

            qb_v = qb[:].rearrange("p (b h w) -> p b h w", b=2, w=QW)
            obv = obT[:].rearrange("p (b r w) -> p b r w", b=2, w=QW)

            sched = {"lgp": None, "exp": None}
            pend_av = []     # (b, kt, gidx, ex_tile) AV lag-1 queue
            o5t = {}         # granule idx -> o5 psum tile

            def emit_exp(b, g, kt, gidx):
                lg = sched["lgp"].tile([128, G], f32, tag="lg", name="lg")
                q0 = b * NQ + g * G
                for j in range(2):
                    nc.tensor.matmul(
                        lg[:, 512 * j:512 * (j + 1)],
                        kvTb[0:4, b * NK + 128 * kt:b * NK + 128 * (kt + 1)],
                        qb[:, q0 + 512 * j:q0 + 512 * (j + 1)],
                        start=True, stop=True,
                    )
                ext = sched["exp"].tile([128, G], bf16, tag="ex", name="ex")
                if pick(C_EXP_A, C_EXP_D) == "A":
                    nc.scalar.activation(ext[:], lg[:], AF.Exp)
                else:
                    nc.vector.tensor_scalar(
                        out=ext[:].bitcast(i16), in0=lg[:],
                        scalar1=SCH_A, scalar2=SCH_B,
                        op0=ALU.mult, op1=ALU.add,
                    )
                pend_av.append((b, kt, gidx, ext))
                if len(pend_av) > 1:
                    emit_av(*pend_av.pop(0))

            def emit_av(b, kt, gidx, ext):
                o5 = o5t[gidx]
                for qc in range(NQC):
                    nc.tensor.matmul(
                        o5[:, 5 * qc:5 * qc + 5],
                        ext[:, qc * QC:(qc + 1) * QC],
                        vp[:, (b * NKT + kt) * 5:(b * NKT + kt + 1) * 5],
                        start=(kt == 0), stop=(kt == NKT - 1),
                        skip_group_check=True,
                    )

            def drain_avs():
                while pend_av:
                    emit_av(*pend_av.pop(0))

            def emit_norm(b, g, gidx):
                o5 = o5t[gidx]
                o5v = o5[:].rearrange("p (q c) -> p q c", c=5)
                dreg = 8 * (gidx % 2)
                dnv = dn[:, dreg:dreg + 8].rearrange("p (q c) -> p q c", c=1)
                nc.vector.reciprocal(dnv, o5v[:, :, 4:5])
                for qc in range(NQC):
                    dst = o4nb[b][:, 32 * g + 4 * qc:32 * g + 4 * qc + 4]
                    src = o5[:, 5 * qc:5 * qc + 4]
                    dcol = dn[:, dreg + qc:dreg + qc + 1]
                    if pick(C_NORM_A, C_NORM_D) == "A":
                        nc.scalar.mul(dst, src, dcol)
                    else:
                        nc.vector.tensor_scalar(
                            out=dst, in0=src, scalar1=dcol,
                            scalar2=None, op0=ALU.mult)

            def emit_taps(b, g, r_lo, r_hi, queues):
                qi = 0
                for dh in range(3):
                    for dw in range(3):
                        t = 3 * dh + dw
                        rr0 = max(r_lo, 1) if dh == 2 else r_lo
                        nr = r_hi - rr0
                        if nr <= 0:
                            continue
                        off = OOF + (1 + rr0 - dh) * WP + QW * b + 1 - dw
                        dst = oo36[4 * t:4 * t + 4, off:off + nr * WP]
                        dst = dst.rearrange(
                            "p (r w) -> p r w", w=WP)[:, :, 0:QW]
                        queues[qi % len(queues)].dma_start(
                            dst, obv[:, b, rr0:rr0 + nr])
                        qi += 1

            ROW_END = [(g + 1) * G // QW for g in range(NG)]  # 42, 85, 128

            def emit_flush(b, g, st):
                # full-o4nb XBAR transpose; only fresh rows consumed
                nc.sync.dma_start_transpose(st[:], o4nb[b][:])
                dst = obT[0:4, b * NQ + g * G:b * NQ + (g + 1) * G]
                dstv = dst.rearrange("p (c q) -> p c q", q=QC)
                for d in range(4):
                    nc.sync.dma_start(
                        dstv[d:d + 1], st[32 * g + d:32 * g + 32:4, 0:QC])
                r_lo = ROW_END[g - 1] if g > 0 else 0
                emit_taps(b, g, r_lo, ROW_END[g], [nc.gpsimd])

            with tc.tile_pool(name="o5p", bufs=2, space="PSUM") as o5p:
                o5t[0] = o5p.tile([128, 5 * NQC], f32, tag="o5", name="o5")
                o5t[1] = o5p.tile([128, 5 * NQC], f32, tag="o5", name="o5")

                # ---- conv + early attention ----
                with (
                    tc.tile_pool(name="cvp", bufs=1) as cvp,
                    tc.tile_pool(name="lg0p", bufs=2, space="PSUM") as lg0p,
                    tc.tile_pool(name="cps", bufs=2, space="PSUM") as cps,
                    tc.tile_pool(name="exp0", bufs=6) as exp0,
                ):
                    sched["lgp"] = lg0p
                    sched["exp"] = exp0
                    xx = cvp.tile([128, PADN], bf16)
                    xx2 = cvp.tile([128, PADN], bf16)
                    qkvT = cvp.tile([12, NPIX], bf16)
                    xx_ap = xx_d.ap()
                    xx2_ap = xx2_d.ap()
                    for q4 in range(4):
                        s4 = (PADN // 4) * q4
                        e4 = PADN if q4 == 3 else (PADN // 4) * (q4 + 1)
                        nc.sync.dma_start(xx[:, s4:e4], xx_ap[:, s4:e4])
                        nc.scalar.dma_start(xx2[:, s4:e4], xx2_ap[:, s4:e4])

                    # dummy matmuls burn the cost model's 3us p-state ramp
                    # during the DMA-in window (result unused)
                    nc.vector.memset(pewarm[:], 1.0)
                    wlg = lg0p.tile([128, G], f32, tag="lg", name="lg")
                    for _ in range(5):
                        nc.tensor.matmul(wlg[0:DH, 0:512], pewarm[:, 0:DH],
                                         pewarm[:], start=True, stop=True)

                    qk_v = qkvT[:].rearrange("p (h w) -> p h w", w=W)
                    qT_v = qkvT[0:4, :].rearrange("p (h w) -> p h w", w=W)

                    # early-exp schedule: group gi (kt 8gi..8gi+8, both
                    # blocks) available from chunk 4gi+5
                    early = {ci: [] for ci in range(NCHUNK)}
                    issued = {(0, 0): set(), (1, 0): set()}
                    pool_tiles = []
                    for kt in range(NKT):
                        for b in range(2):
                            pool_tiles.append((b, kt))
                    pti = 0
                    for ci in range(5, NCHUNK):
                        cap = 2 if ci < 12 else 4
                        while cap > 0 and pti < len(pool_tiles):
                            b, kt = pool_tiles[pti]
                            if ci < 4 * (kt // 8) + 5:
                                break
                            early[ci].append((b, kt))
                            issued[(b, 0)].add(kt)
                            pti += 1
                            cap -= 1

                    for ci in range(NCHUNK):
                        ps = cps.tile([12, CN], f32, tag="cps")
                        f0 = ci * CHUNK_ROWS * WP
                        for dh in range(3):
                            nc.tensor.matmul(
                                ps[:], wqkv[:, 12 * dh:12 * (dh + 1)],
                                xx[:, f0 + dh * WP:f0 + dh * WP + CN],
                                start=(dh == 0), stop=False,
                            )
                        nc.tensor.matmul(
                            ps[:], wqkv[:, 36:48], xx2[:, f0:f0 + CN],
                            start=False, stop=False,
                        )
                        nc.tensor.matmul(
                            ps[:], wqkv[0:CIN, 48:60],
                            xx[0:CIN, f0 + 2 * WP + 2:f0 + 2 * WP + 2 + CN],
                            start=False, stop=True,
                        )
                        # bias add + junk-column drop (cast to bf16)
                        psv = ps[:].rearrange("p (r c) -> p r c", c=WP)
                        dst = qkvT[:, ci * CHUNK_ROWS * W:
                                   (ci + 1) * CHUNK_ROWS * W]
                        if pick(C_DRAIN_A, C_DRAIN_D) == "A":
                            nc.scalar.add(dst, psv[:, :, 0:W], bias12[:])
                        else:
                            nc.vector.tensor_scalar_add(
                                dst, psv[:, :, 0:W], bias12[:])
                        r0 = ci * CHUNK_ROWS
                        rr = slice(r0, r0 + CHUNK_ROWS)
                        for b in range(2):
                            nc.gpsimd.tensor_copy(
                                qb_v[:, b, rr],
                                qT_v[:, rr, QW * b:QW * b + QW])
                        if ci % 4 == 3:
                            gi = ci // 4
                            rsl = slice(32 * gi, 32 * gi + 32)
                            for b in range(2):
                                dsl = slice(b * NK + 1024 * gi,
                                            b * NK + 1024 * (gi + 1))
                                nc.sync.dma_start(
                                    kvTb[:, dsl],
                                    qk_v[4:12, rsl, 16 * b:16 * b + KW])
                            # V' build: transpose k+v rows at once (base
                            # partition 0), copy the 4 v cols to vp
                            for b in range(2):
                                tg = cps.tile([128, 64], bf16, tag="cps")
                                for j in range(8):
                                    kt = 8 * gi + j
                                    nc.tensor.matmul(
                                        tg[:, 8 * j:8 * j + 8],
                                        kvTb[0:8, b * NK + 128 * kt:
                                             b * NK + 128 * (kt + 1)],
                                        id8[:], is_transpose=True,
                                    )
                                dstv = vp[:].rearrange(
                                    "p (t c) -> p t c", c=5)
                                srcv = tg[:].rearrange(
                                    "p (t c) -> p t c", c=8)
                                d0 = b * NKT + 8 * gi
                                if pick(C_VP_A, C_VP_D) == "A":
                                    nc.scalar.copy(
                                        dstv[:, d0:d0 + 8, 0:4],
                                        srcv[:, :, 4:8])
                                else:
                                    nc.vector.tensor_copy(
                                        dstv[:, d0:d0 + 8, 0:4],
                                        srcv[:, :, 4:8])
                        for b, kt in early[ci]:
                            emit_exp(b, 0, kt, b)

                # ---- steady attention ----
                order = [(0, 0), (1, 0), (0, 1), (1, 1), (0, 2), (1, 2)]
                with (
                    tc.tile_pool(name="lgp", bufs=3, space="PSUM") as lgp,
                    tc.tile_pool(name="expp", bufs=6) as expp,
                    tc.tile_pool(name="stp", bufs=2) as stp,
                ):
                    sched["lgp"] = lgp
                    sched["exp"] = expp
                    for gidx, (b, g) in enumerate(order):
                        if gidx >= 2:
                            o5t[gidx] = o5p.tile([128, 5 * NQC], f32,
                                                 tag="o5", name="o5")
                        done = issued.get((b, g), set())
                        for kt in range(NKT):
                            if kt in done:
                                continue
                            emit_exp(b, g, kt, gidx)
                        drain_avs()
                        emit_norm(b, g, gidx)
                        if gidx < 5:
                            st = stp.tile([128, 128], bf16, tag="st")
                            emit_flush(b, g, st)
                        else:
                            # tail flush: PE transpose + engine copy (lower
                            # latency than XBAR), shifts/taps spread over
                            # the DMA queues
                            stt = stp.tile([128, 128], bf16, tag="st")
                            tg2 = lgp.tile([128, G], f32, tag="lg",
                                           name="lg")
                            tg2v = tg2[0:32, 0:64].bitcast(bf16)
                            nc.tensor.matmul(
                                tg2v, o4nb[1][:, 64:96], id128[:],
                                is_transpose=True)
                            if pick(300.0, 260.0) == "A":
                                nc.scalar.copy(stt[0:32, :], tg2v)
                            else:
                                nc.vector.tensor_copy(stt[0:32, :], tg2v)
                            dst = obT[0:4, NQ + 2 * G:NQ + 3 * G]
                            dstv = dst.rearrange("p (c q) -> p c q", q=QC)
                            tailq = [nc.sync, nc.scalar, nc.gpsimd]
                            for d in range(4):
                                tailq[d % len(tailq)].dma_start(
                                    dstv[d:d + 1], stt[d:32:4, 0:QC])
                            emit_taps(1, 2, ROW_END[1], ROW_END[2], tailq)

            # ---- output conv: one matmul per chunk over the 36-row stack
            outp_ap = outp_d.ap()
            with (
                tc.tile_pool(name="ops", bufs=6, space="PSUM") as ops,
                tc.tile_pool(name="ost", bufs=3) as ost,
            ):
                for c2 in range(NCHUNK // 2):
                    stage = ost.tile([CIN, 2 * CHUNK_ROWS * W], f32,
                                     tag="ost")
                    for half in range(2):
                        ci = 2 * c2 + half
                        ps = ops.tile([CIN, CN], f32, tag="ops")
                        f0 = OOF + ci * CHUNK_ROWS * WP
                        nc.tensor.matmul(
                            ps[:], wo36[:], oo36[:, f0:f0 + CN],
                            start=True, stop=True,
                        )
                        psv = ps[:].rearrange("p (r c) -> p r c", c=WP)
                        dst = stage[:, half * CHUNK_ROWS * W:
                                    (half + 1) * CHUNK_ROWS * W]
                        if pick(C_OC_A, C_OC_D) == "A":
                            nc.scalar.copy(dst, psv[:, :, 0:W])
                        else:
                            nc.vector.tensor_copy(dst, psv[:, :, 0:W])
                    nc.sync.dma_start(
                        outp_ap[:, 2 * c2 * CHUNK_ROWS * W:
                                (2 * c2 + 2) * CHUNK_ROWS * W],
                        stage[:])

    nc.compile()
    return nc


def _prep_inputs(x, wq, bq, wk, bk, wv, bv, wo):
    f32 = np.float32
    x = np.ascontiguousarray(np.asarray(x, f32))
    scale = f32(DH) ** -0.5

    bf = ml_bf16()
    xx = np.zeros((128, PADN), np.float32)
    xv = xx[:CIN, :HP * WP].reshape(CIN, HP, WP)
    xv[:, 1:1 + H, 1:1 + W] = x[0].transpose(2, 0, 1)
    xx[CIN:, :PADN - 1] = xx[:CIN, 1:]
    xx2 = np.zeros((128, PADN), np.float32)
    xx2[:CIN, :PADN - 2] = xx[:CIN, 2:]
    xx2[CIN:, :PADN - (WP + 2)] = xx[:CIN, WP + 2:]
    xx2 = xx2.astype(bf)
    xx = xx.astype(bf)

    wq = np.asarray(wq, f32) * scale
    bq = np.asarray(bq, f32) * scale
    wk = np.asarray(wk, f32)
    bk = np.asarray(bk, f32)
    wv = np.asarray(wv, f32)
    bv = np.asarray(bv, f32)
    wo = np.asarray(wo, f32)

    id8 = np.eye(8, dtype=bf)
    id128 = np.eye(128, dtype=bf)
    in_maps = []
    for h in range(NH):
        sl = slice(4 * h, 4 * h + 4)
        wqkv = np.zeros((128, 5, 12), f32)
        for dh in range(3):
            for p, dw in ((0, 0), (1, 1)):   # pair slots on partition halves
                wqkv[64 * p:64 * p + CIN, dh, 0:4] = wq[dh, dw, :, sl]
                wqkv[64 * p:64 * p + CIN, dh, 4:8] = wk[dh, dw, :, sl]
                wqkv[64 * p:64 * p + CIN, dh, 8:12] = wv[dh, dw, :, sl]
        for p, dh in ((0, 0), (1, 1)):       # (0,2)+(1,2) pair on xx2 halves
            wqkv[64 * p:64 * p + CIN, 3, 0:4] = wq[dh, 2, :, sl]
            wqkv[64 * p:64 * p + CIN, 3, 4:8] = wk[dh, 2, :, sl]
            wqkv[64 * p:64 * p + CIN, 3, 8:12] = wv[dh, 2, :, sl]
        wqkv[:CIN, 4, 0:4] = wq[2, 2, :, sl]
        wqkv[:CIN, 4, 4:8] = wk[2, 2, :, sl]
        wqkv[:CIN, 4, 8:12] = wv[2, 2, :, sl]
        bias12 = np.concatenate([bq[sl], bk[sl], bv[sl]]).reshape(12, 1)
        wo36 = np.zeros((36, 64), f32)
        for dh in range(3):
            for dw in range(3):
                t = 3 * dh + dw
                wo36[4 * t:4 * t + 4, :] = wo[dh, dw, sl, :]
        in_maps.append({
            "xx": xx,
            "bias12": np.ascontiguousarray(bias12.astype(f32)),
            "wqkv": np.ascontiguousarray(wqkv.reshape(128, 5 * 12).astype(bf)),
            "xx2": xx2,
            "wo36": np.ascontiguousarray(wo36.astype(bf)),
            "id8": id8,
            "id128": id128,
        })
    return in_maps


def ml_bf16():
    import ml_dtypes
    return ml_dtypes.bfloat16


def _run(in_maps, trace=False, trace_cores=None):
    from concourse.bass_utils import run_bass_kernel_spmd

    if "nc" not in _cached:
        _cached["nc"] = _build_nc()
    return run_bass_kernel_spmd(
        _cached["nc"], in_maps, core_ids=list(range(NH)),
        trace=trace, trace_cores=trace_cores,
    )


def kernel(x, wq, bq, wk, bk, wv, bv, wo):
    in_maps = _prep_inputs(x, wq, bq, wk, bk, wv, bv, wo)
    res = _run(in_maps)
    acc = np.zeros((CIN, NPIX), np.float64)
    for r in res.results:
        acc += r["outp"].astype(np.float64)
    out = acc.astype(np.float32).reshape(CIN, H, W).transpose(1, 2, 0)
    return out[None]
